# revision 22
# baseline (speedup 1.0000x reference)
"""Trainium2 Bass kernel for the ConvFeatureExtractor problem.

Reference computation (all f32):
    matches[f, i] = sum_j kmer_params[f, kmer_idcs[i, j], j]      # (F, M)
    probs = softmax(matches / temperature, axis=1)                # over M
    pooled = freq @ probs.T                                       # (B, F)
    profile = pooled / pooled.sum(axis=1, keepdims=True)

Shapes: B=1024, M=4096 (=4^6 kmers), F=8192 filters, K=6, 4 bases.
Filter-sharded across 8 cores (FL = 1024 filters/core).

Schedule (from trace iteration; 285941 -> ~210000 ns):
  * PE warm-up matmuls run during the input-DMA ramp so HAM is at
    K=8/8 when the real stream starts.
  * E = exp(matches/T): K=24 matmuls 2-way row-group-packed
    (tile_position (0,0)/(32,0), onehot/params replicated to partition
    32 on host); one (128,1024) PSUM tile (2 banks) and ONE exp per
    k-tile keeps the ACT chain at its ~37us floor; Z accumulated on
    DVE behind the exps.
  * U = freq @ E^T: 8 accumulator chains (128,1024 = 2 banks each,
    double-buffered), drained by ACT copies (f32); 1/Z applied later
    on DVE off the bank-critical path (last tile: fused drain*1/Z).
  * Row-sum s is computed WITHOUT the U results so the AllReduce can
    hide under the U matmuls: c[i] = sum_f E[i,f]/Z[f] via
    GPSIMD/DVE-split muls + batched 3D-AP DVE reduces, broadcast to
    crep on ACT, then s = crep^T @ freq (64 high-priority matmuls,
    ~130us).  4KB AllReduce (~9-29us) completes ~25us before the U
    stream ends.
  * finalize: 1/s broadcast to partitions via K=1 matmuls against
    ones, then out[b] = U_sb * rinv[b] per-partition scale (DVE/ACT
    split), written bf16 (host converts to f32 and concatenates).
"""

import os

import numpy as np
import ml_dtypes

import concourse.bass as bass  # noqa: F401
import concourse.tile as tile
from concourse import bacc, mybir
from concourse.bass_utils import run_bass_kernel_spmd

NCORES = 8
B = 1024           # batch
M = 4096           # 4^6 kmers
F = 8192           # filters
KMER = 6           # kmer length
NBASE = 4
KK = NBASE * KMER  # 24 flattened (base, position)
FL = F // NCORES   # 1024 filters per core

MT = M // 128      # 32 contraction tiles
BT = B // 128      # 8 batch tiles
FC = 512           # psum free chunk
NFC = FL // FC     # 2

BF16 = mybir.dt.bfloat16
F32 = mybir.dt.float32
AFT = mybir.ActivationFunctionType
ALU = mybir.AluOpType

_CACHE: dict = {}


def _body(tc, freqT, ohrep, parrep, tempr, out):
    nc = tc.nc
    no_tilepos = os.environ.get("KERNEL_NO_TILEPOS", "")
    with (
        tc.tile_pool(name="res", bufs=1) as res,
        tc.tile_pool(name="upool", bufs=2, space="PSUM") as up,
        tc.tile_pool(name="outp", bufs=2) as outp,
        tc.tile_pool(name="dram", bufs=1, space="DRAM") as dram,
    ):
        # ---------- small inputs / constants ----------
        oh_sb = res.tile([64, M], BF16)
        nc.sync.dma_start(oh_sb[:], ohrep[:])
        par_sb = res.tile([64, FL], BF16)
        nc.sync.dma_start(par_sb[:], parrep[:])
        t_sb = res.tile([128, 1], F32)
        nc.sync.dma_start(t_sb[:], tempr[:])
        invt = res.tile([128, 1], F32)
        nc.vector.reciprocal(invt[:], t_sb[:])
        ones_bf = res.tile([128, 128], BF16)
        nc.vector.memset(ones_bf[:], 1.0)
        ones_f = res.tile([1, 128], F32)
        nc.vector.memset(ones_f[:], 1.0)

        # ---------- stream in freq^T (M, B) as 32 k-tiles ----------
        freq_sb = res.tile([128, MT * B], BF16)
        for k in range(MT):
            nc.sync.dma_start(freq_sb[:, k * B:(k + 1) * B],
                              freqT[k * 128:(k + 1) * 128, :])

        # PE warm-up: dense dummy matmuls while the input DMAs land, so
        # HAM un-throttles (K=8/8) before the real matmul stream starts.
        if not os.environ.get("KERNEL_NO_WARMUP", ""):
            with tc.tile_pool(name="wpool", bufs=1, space="PSUM") as wp:
                wps = wp.tile([128, FC], F32, tag="wps")
                for _ in range(64):
                    nc.tensor.matmul(wps[:, 0:128], lhsT=ones_bf[:],
                                     rhs=ones_bf[:], start=True, stop=True)

        E_sb = res.tile([128, MT * FL], BF16)
        zacc = res.tile([128, FL], F32)
        nc.vector.memset(zacc[:], 0.0)
        invz_bc = res.tile([128, FL], BF16)

        # ---------- E = exp(matches/T), Z accumulation ----------
        # one (128,1024) PSUM tile per k (spans 2 banks, 2 matmuls), ONE
        # exp per k so the ACT chain runs at its 37us floor; bufs=2 so
        # the next k's matmuls never wait on the previous exp.
        with tc.tile_pool(name="epool", bufs=2, space="PSUM") as ep:
            for k in range(MT):
                pm = ep.tile([128, FL], F32, tag="pm")
                ks = slice(k * 128, (k + 1) * 128)
                if no_tilepos:
                    nc.tensor.matmul(pm[:, 0:FC], lhsT=oh_sb[0:KK, ks],
                                     rhs=par_sb[0:KK, 0:FC],
                                     start=True, stop=True)
                    nc.tensor.matmul(pm[:, FC:FL], lhsT=oh_sb[0:KK, ks],
                                     rhs=par_sb[0:KK, FC:FL],
                                     start=True, stop=True)
                else:
                    nc.tensor.matmul(pm[:, 0:FC], lhsT=oh_sb[0:KK, ks],
                                     rhs=par_sb[0:KK, 0:FC],
                                     start=True, stop=True,
                                     tile_position=(0, 0))
                    nc.tensor.matmul(pm[:, FC:FL], lhsT=oh_sb[32:32 + KK, ks],
                                     rhs=par_sb[32:32 + KK, FC:FL],
                                     start=True, stop=True,
                                     tile_position=(32, 0))
                nc.scalar.activation(E_sb[:, k * FL:(k + 1) * FL],
                                     pm[:], AFT.Exp, scale=invt[:])
                nc.vector.tensor_add(zacc[:], zacc[:],
                                     E_sb[:, k * FL:(k + 1) * FL])

        # ---------- Z -> 1/Z broadcast to all partitions ----------
        zacc_bf = res.tile([128, FL], BF16)
        with tc.high_priority():
            nc.vector.tensor_copy(zacc_bf[:], zacc[:])
            with tc.tile_pool(name="zpool", bufs=1, space="PSUM") as zp:
                for fc in range(NFC):
                    zps = zp.tile([128, FC], F32, tag="zps")
                    nc.tensor.matmul(zps[:], lhsT=ones_bf[:],
                                     rhs=zacc_bf[:, fc * FC:(fc + 1) * FC],
                                     start=True, stop=True)
                    with nc.allow_low_precision(
                            reason="1/Z stored bf16; softmax weights "
                                   "tolerate 0.4% and share values with c"):
                        nc.vector.reciprocal(
                            invz_bc[:, fc * FC:(fc + 1) * FC], zps[:])

        # ---------- c[i] = sum_f E[i,f]/Z[f]  (GPSIMD+DVE muls, batched
        # DVE reduces, crep broadcast on ACT).  Emitted BEFORE the U loop
        # so this work starts as soon as invz_bc is ready (~48us) and the
        # s matmuls + AllReduce hide under the U matmuls.
        invz2 = res.tile([128, 2 * FL], BF16)
        nc.vector.tensor_copy(invz2[:, 0:FL], invz_bc[:])
        nc.vector.tensor_copy(invz2[:, FL:2 * FL], invz_bc[:])
        ctmp = res.tile([128, 4 * FL], BF16)
        c_col = res.tile([128, MT], F32)
        crep = res.tile([128, MT * 64], BF16)
        use_gps = not os.environ.get("KERNEL_NO_GPSIMD", "")
        batch_red = not os.environ.get("KERNEL_NO_BATCHRED", "")
        with tc.high_priority():
            for kp in range(MT // 2):        # pairs of k-tiles
                tslot = ctmp[:, (kp % 2) * 2 * FL:((kp % 2) + 1) * 2 * FL]
                eng = nc.gpsimd if (use_gps and kp % 3 == 0) else nc.vector
                eng.tensor_mul(tslot, E_sb[:, 2 * kp * FL:(2 * kp + 2) * FL],
                               invz2[:])
                if batch_red:
                    nc.vector.reduce_sum(
                        c_col[:, 2 * kp:2 * kp + 2],
                        tslot.rearrange("p (n f) -> p n f", n=2),
                        axis=mybir.AxisListType.X)
                else:
                    for j in range(2):
                        nc.vector.reduce_sum(
                            c_col[:, 2 * kp + j:2 * kp + j + 1],
                            tslot[:, j * FL:(j + 1) * FL],
                            axis=mybir.AxisListType.X)
            for k in range(MT):
                nc.scalar.activation(crep[:, k * 64:(k + 1) * 64],
                                     ones_bf[:, 0:64], AFT.Copy,
                                     scale=c_col[:, k:k + 1])

        # ---------- U = freq @ E^T; one ACT drain per b (2 banks) --------
        # s_dup = crep^T @ freq matmuls are emitted inline (after b=3 and
        # b=5) so they land mid-stream at natural priority: late enough
        # that crep is ready (no PE stall), early enough that the
        # AllReduce still hides under the remaining U matmuls.
        U_sb = res.tile([128, BT * FL], F32)
        s_sb = res.tile([1, B], F32)
        with tc.tile_pool(name="spool", bufs=1, space="PSUM") as sp:
            sA = sp.tile([64, FC], F32, tag="sA")
            sB = sp.tile([64, FC], F32, tag="sB")

            for b in range(BT):
                uf = up.tile([128, FL], F32, tag="uf")
                for k in range(MT):
                    lw = freq_sb[:, k * B + b * 128: k * B + (b + 1) * 128]
                    nc.tensor.matmul(uf[:, 0:FC], lhsT=lw,
                                     rhs=E_sb[:, k * FL: k * FL + FC],
                                     start=(k == 0), stop=(k == MT - 1))
                    nc.tensor.matmul(uf[:, FC:FL], lhsT=lw,
                                     rhs=E_sb[:, k * FL + FC: (k + 1) * FL],
                                     start=(k == 0), stop=(k == MT - 1))
                if b == BT - 1:
                    # last tile: fuse drain + 1/Z (tail-critical, its bank
                    # has no successor waiting)
                    with tc.high_priority():
                        nc.vector.tensor_mul(U_sb[:, b * FL:(b + 1) * FL],
                                             uf[:], invz_bc[:])
                else:
                    nc.scalar.copy(U_sb[:, b * FL:(b + 1) * FL], uf[:])

            with tc.high_priority():
                for k in range(MT):
                    lwc = crep[:, k * 64:(k + 1) * 64]
                    nc.tensor.matmul(sA[:], lhsT=lwc,
                                     rhs=freq_sb[:, k * B: k * B + FC],
                                     start=(k == 0), stop=(k == MT - 1))
                    nc.tensor.matmul(sB[:], lhsT=lwc,
                                     rhs=freq_sb[:, k * B + FC:
                                                 k * B + 2 * FC],
                                     start=(k == 0), stop=(k == MT - 1))
                nc.scalar.copy(s_sb[0:1, 0:FC], sA[0:1, :])
                nc.scalar.copy(s_sb[0:1, FC:B], sB[0:1, :])

        # 1/Z applied in-place on U_sb (off the PSUM-bank critical path)
        for b in range(BT - 1):
            nc.vector.tensor_mul(U_sb[:, b * FL:(b + 1) * FL],
                                 U_sb[:, b * FL:(b + 1) * FL], invz_bc[:])

        # ---------- AllReduce s over the 8 cores (4KB) ----------
        ssum_sb = res.tile([1, B], F32)
        if os.environ.get("KERNEL_NO_COLLECTIVE"):
            nc.vector.tensor_scalar_mul(ssum_sb[:], s_sb[:], float(NCORES))
        else:
            s_in = dram.tile([1, B], F32)
            s_out = dram.tile([1, B], F32, addr_space="Shared")
            nc.sync.dma_start(s_in[:], s_sb[:])
            nc.gpsimd.collective_compute(
                "AllReduce", ALU.add,
                replica_groups=[list(range(NCORES))],
                ins=[s_in.opt()], outs=[s_out.opt()])
            nc.sync.dma_start(ssum_sb[:], s_out[:])
        # ---------- broadcast s to partitions via K=1 matmuls, then 1/x ----
        # high priority: must run right after the AllReduce lands, NOT at
        # the tail of the PE queue, so finalize overlaps remaining U MMs.
        rcol = res.tile([128, BT], F32)
        with tc.tile_pool(name="rpool", bufs=2, space="PSUM") as rp, \
                tc.high_priority():
            for j in range(BT):
                rb = rp.tile([128, 128], F32, tag="rb")
                nc.tensor.matmul(rb[:], lhsT=ssum_sb[0:1, j * 128:(j + 1) * 128],
                                 rhs=ones_f[0:1, :], start=True, stop=True)
                nc.scalar.copy(rcol[:, j:j + 1], rb[:, 0:1])
        with tc.high_priority():
            nc.vector.reciprocal(rcol[:], rcol[:])

        # ---------- profile = U_sb * (1/s); write out bf16 ----------
        with tc.high_priority():
            for b in range(BT):
                prof = outp.tile([128, FL], BF16, tag="prof")
                if b % 2 == 0:
                    nc.vector.tensor_scalar_mul(prof[:],
                                                U_sb[:, b * FL:(b + 1) * FL],
                                                rcol[:, b:b + 1])
                else:
                    nc.scalar.activation(prof[:], U_sb[:, b * FL:(b + 1) * FL],
                                         AFT.Copy, scale=rcol[:, b:b + 1])
                nc.sync.dma_start(out[b * 128:(b + 1) * 128, :], prof[:])


def _build_bass():
    nc = bacc.Bacc("TRN2", target_bir_lowering=False, debug=False,
                   num_devices=NCORES)
    freqT = nc.dram_tensor("freqT", [M, B], BF16, kind="ExternalInput").ap()
    ohrep = nc.dram_tensor("ohrep", [64, M], BF16, kind="ExternalInput").ap()
    parrep = nc.dram_tensor("parrep", [64, FL], BF16, kind="ExternalInput").ap()
    tempr = nc.dram_tensor("tempr", [128, 1], F32, kind="ExternalInput").ap()
    out = nc.dram_tensor("out", [B, FL], BF16, kind="ExternalOutput").ap()

    with tile.TileContext(nc) as tc:
        _body(tc, freqT, ohrep, parrep, tempr, out)
    nc.compile()
    return nc


def _get_nc():
    if "nc" not in _CACHE:
        _CACHE["nc"] = _build_bass()
    return _CACHE["nc"]


def _prepare_in_maps(freq, kmer_params, temperature, kmer_idcs):
    freq = np.asarray(freq, dtype=np.float32)            # (B, M)
    kp = np.asarray(kmer_params, dtype=np.float32)       # (F, 4, K)
    temp = np.asarray(temperature, dtype=np.float32).reshape(-1)[:1]
    idcs = np.asarray(kmer_idcs).astype(np.int64)        # (M, K)

    assert freq.shape == (B, M) and kp.shape == (F, NBASE, KMER)
    assert idcs.shape == (M, KMER)

    # one-hot re-encoding of the index input, replicated to row groups
    # 0 and 32 for the 2-way row-tiled matmul
    onehot = np.zeros((M, NBASE, KMER), dtype=np.float32)
    onehot[np.arange(M)[:, None], idcs, np.arange(KMER)[None, :]] = 1.0
    ohT = np.ascontiguousarray(onehot.reshape(M, KK).T)  # (24, M)
    ohrep = np.zeros((64, M), dtype=np.float32)
    ohrep[0:KK] = ohT
    ohrep[32:32 + KK] = ohT
    ohrep = ohrep.astype(ml_dtypes.bfloat16)

    params_flat = kp.reshape(F, KK)
    freqT = np.ascontiguousarray(freq.T).astype(ml_dtypes.bfloat16)
    tempr = np.ascontiguousarray(np.broadcast_to(temp.reshape(1, 1), (128, 1)))

    in_maps = []
    for c in range(NCORES):
        pT = np.ascontiguousarray(
            params_flat[c * FL:(c + 1) * FL].T)          # (24, FL)
        prep = np.zeros((64, FL), dtype=np.float32)
        prep[0:KK] = pT
        prep[32:32 + KK] = pT
        in_maps.append({
            "freqT": freqT,
            "ohrep": ohrep,
            "parrep": prep.astype(ml_dtypes.bfloat16),
            "tempr": tempr,
        })
    return in_maps


def _run(in_maps, trace=False):
    nc = _get_nc()
    return run_bass_kernel_spmd(nc, in_maps, list(range(NCORES)), trace=trace)


def kernel(freq, kmer_params, temperature, kmer_idcs):
    in_maps = _prepare_in_maps(freq, kmer_params, temperature, kmer_idcs)
    res = _run(in_maps,
               trace=os.environ.get("KERNEL_TRACE", "") not in ("", "0"))
    _CACHE["last_result"] = res
    return np.concatenate(
        [np.asarray(res.results[c]["out"]).astype(np.float32)
         for c in range(NCORES)], axis=1)
